# revision 56
# baseline (speedup 1.0000x reference)
"""CompressionAttention Trainium2 kernel (8 NeuronCores, SPMD).

Sharding: core i handles batch b=i//4 and 4 heads hh=i%4 (model-dim slice
hh*256:(hh+1)*256). Heads never interact before out_proj, so each core
computes a partial out-projection for its batch; the host gather sums the
4 partials per batch and adds bo once.

Algorithm per core (chunked linear attention, chunk T=128):
  w[c,t] = exp(qc_c . k_t)            (max-subtraction dropped: att is
                                       invariant to per-c scaling of w)
  den[c,s]   = cumsum_t<=s w[c,t]
  att[c,s]   = (w_chunk^T (U o G) + SK^T qs) / den   per chunk + state
  p = softmax_c att ; o_s = sum_c p * (sum_t<=s w v)/den

Schedule: all cross-chunk recurrences (den carry, SK/SV prefixes) and all
G = k^T q blocks are precomputed in side phases, so the main chunk loop has
no chunk->chunk dependency and the Tensor engine never idles long enough to
HAM-throttle. Softmax elementwise ops run in a partition-packed (32h+c, t)
layout (engine time ~ free-size, so 4x cheaper than (c, 4h*t)); the packed
e/pdd are re-laid-out for their matmul consumers with tiny SBUF-to-SBUF
DMAs. 1/x runs as exp(-ln(x)) on Act (single act table, den carry folded
into the ln bias); 1/sums o-normalization is deferred out of the loop.
"""
import os
import numpy as np
import ml_dtypes

import concourse.bacc as bacc
import concourse.tile as tile
from concourse import mybir
from concourse.bass_utils import run_bass_kernel_spmd

F32 = mybir.dt.float32
BF16 = mybir.dt.bfloat16
EXP = mybir.ActivationFunctionType.Exp
LN = mybir.ActivationFunctionType.Ln
IDENT = mybir.ActivationFunctionType.Identity

S, B, D = 2048, 2, 1024
H, DH, C = 16, 64, 16
HPC = 4            # heads per core
HD = HPC * DH      # 256 model dims per core
T = 128            # chunk
NCH = S // T       # 16 chunks
KT = D // 128      # 8 k-tiles of the model dim

LAST_EXEC_NS = None
_CACHE = {}


def _patched_insert_act_table_loads(self):
    """Force every activation (Exp/Ln/Copy/Identity) onto the single
    act-func table that contains them all. The default greedy pass picks
    exp_and_others for Exp and natural_log_exp_and_others for Ln, inserting
    an ACT_TABLE_LOAD (1.28us!) on every switch."""
    import bass_rust as _bass_rust
    from concourse.hw_specs import get_activation_tables
    has_activation = any(
        isinstance(i, mybir.InstActivation)
        for b in self.main_func.blocks
        for i in b.instructions)
    if not has_activation:
        return
    keep = {mybir.ActivationFunctionType.Exp, mybir.ActivationFunctionType.Ln,
            mybir.ActivationFunctionType.Copy,
            mybir.ActivationFunctionType.Identity}
    tables = []
    for name, funcs in get_activation_tables(self.m.arch).items():
        if name != "natural_log_exp_and_others":
            funcs = funcs - keep
        tables.append((name, funcs))
    _bass_rust.insert_act_table_loads(self, tables)


def _build():
    nc = bacc.Bacc("TRN2", target_bir_lowering=False, debug=False, num_devices=8)
    import types as _types
    nc.insert_act_table_loads = _types.MethodType(
        _patched_insert_act_table_loads, nc)

    # ---- DRAM I/O (per-core, host-prepped layouts; bf16 where possible).
    # x and weights are pre-shuffled on host so each SBUF partition line is
    # ONE contiguous DRAM run (8KB / 4KB descriptors -> near-peak DMA bw;
    # the natural (kt p) layouts gave 512B-1KB descriptors, ~3x slower). ----
    xT_d = nc.dram_tensor("xT", [128, 4, KT, 512], BF16, kind="ExternalInput")
    wqT_d = nc.dram_tensor("wqT", [128, KT, HD], BF16, kind="ExternalInput")
    wkT_d = nc.dram_tensor("wkT", [128, KT, HD], BF16, kind="ExternalInput")
    wvT_d = nc.dram_tensor("wvT", [128, KT, HD], BF16, kind="ExternalInput")
    woT_d = nc.dram_tensor("woT", [128, 2, D], BF16, kind="ExternalInput")
    # qc block-diagonal per head PAIR: rows 0-63 = head 2m's d, rows 64-127 =
    # head 2m+1's d (matching the [k_2m; k_2m+1] stacking of kT[m]), cols =
    # (j, c) with zero off-blocks and zero c-pad 16->32 (pad logits are 0 ->
    # w=1, never read). One matmul then emits BOTH heads' wT/w16 block.
    qcb_d = nc.dram_tensor("qcb", [2, 128, 64], BF16, kind="ExternalInput")
    # biases host-prepped as (p, m) so each partition line is one contiguous
    # 8B descriptor (the (m p)->p m rearrange of a flat [256] makes 256
    # 4-byte descriptors).
    bq_d = nc.dram_tensor("bq", [128, 2], F32, kind="ExternalInput")
    bk_d = nc.dram_tensor("bk", [128, 2], F32, kind="ExternalInput")
    out_d = nc.dram_tensor("out_p", [S, D], BF16, kind="ExternalOutput")

    # ---- consts baked into the NEFF ----
    u = np.triu(np.ones((T, T), np.float32))
    u4 = np.tile(u, (1, 4)).astype(ml_dtypes.bfloat16)
    u4_d = nc.inline_tensor(u4, "u4c")
    # sel2[pair][p, j] = 1 iff p selects head 2*pair (j<64) / 2*pair+1
    # (j>=64): the sums-broadcast matmul consumes packed (32h+c) e directly
    # and emits both heads of a column-block in one matmul.
    sel2 = np.zeros((2, 128, 128), np.float32)
    for pair in range(2):
        sel2[pair, 64 * pair:64 * pair + C, 0:64] = 1.0
        sel2[pair, 64 * pair + 32:64 * pair + 32 + C, 64:128] = 1.0
    sel2_d = nc.inline_tensor(
        np.ascontiguousarray(sel2.transpose(1, 0, 2)).astype(ml_dtypes.bfloat16),
        "sel2")
    kv_scratch = [nc.dram_tensor(f"kvs{j}", [128, 2048], BF16) for j in range(2)]

    with tile.TileContext(nc) as tc:
        _emit(nc, tc, locals())
    nc.compile()
    return nc


def _emit(nc, tc, d):
    from contextlib import ExitStack

    with ExitStack() as ctx:
        ep = ctx.enter_context

        # ---------- persistent pools ----------
        consts = ep(tc.tile_pool(name="consts", bufs=1))
        wpool = ep(tc.tile_pool(name="wpool", bufs=1))      # WoT
        qkv = ep(tc.tile_pool(name="qkv", bufs=1))          # qT/kT/vnat
        qkv2 = ep(tc.tile_pool(name="qkv2", bufs=1))        # knat, row-0 copies
        wstore = ep(tc.tile_pool(name="wstore", bufs=1))    # w16/wT/carry/sk/sv
        gstore = ep(tc.tile_pool(name="gstore", bufs=1))    # masked G, all chunks
        onum = ep(tc.tile_pool(name="onum", bufs=1))        # o_num, rb_all
        sb2 = ep(tc.tile_pool(name="sb2", bufs=3))          # per-chunk sbuf
        otp = ep(tc.tile_pool(name="otp", bufs=1))          # oT final
        outp = ep(tc.tile_pool(name="outp", bufs=3))        # out staging

        # ---------- consts (issued on the Act HWDGE sequencer so the SP
        # sequencer's 565ns/issue budget is spent only on the big x/weight
        # loads; Act is idle during the load phase anyway) ----------
        def bias_tile(name):
            t = consts.tile([128, 2], F32, tag=name)
            nc.scalar.dma_start(out=t, in_=d[name + "_d"].ap())
            return t
        bk_t, bq_t = bias_tile("bk"), bias_tile("bq")
        u4_t = consts.tile([128, 4 * T], BF16, tag="u4")
        nc.scalar.dma_start(out=u4_t, in_=d["u4_d"].ap())
        u128 = u4_t[:, 0:T]
        sel2_t = consts.tile([128, 2, 128], BF16, tag="sel2")
        nc.scalar.dma_start(out=sel2_t, in_=d["sel2_d"].ap())
        qcb_t = consts.tile([128, 2, 64], BF16, tag="qcb")
        nc.scalar.dma_start(
            out=qcb_t, in_=d["qcb_d"].ap().rearrange("m p j -> p m j"))

        # WoT resident (bf16): (128, 2, D); only needed at the first oproj
        # block mid-loop, so it loads on Act after the consts.
        woT_t = wpool.tile([128, 2, D], BF16, tag="woT")
        nc.scalar.dma_start(out=woT_t, in_=d["woT_d"].ap())

        # ---------- projections (all bf16) ----------
        qT = [qkv.tile([128, S], BF16, tag=f"qT{m}", name=f"qT{m}") for m in range(2)]
        kT = [qkv.tile([128, S], BF16, tag=f"kT{m}", name=f"kT{m}") for m in range(2)]
        vnat = qkv.tile([128, NCH, HD], BF16, tag="vnat", name="vnat")

        with tc.tile_pool(name="xw", bufs=1) as xw, \
                tc.tile_pool(name="ppj", bufs=4, space="PSUM") as ppj:
            # DMA order = consumption order, ONE dma_start per logical block
            # (descriptors spray across all 16 DMA engines, so a block moves
            # at aggregate bandwidth; splitting it only costs SP issue slots
            # at 565ns each). x goes s-block-major: the n-outer wave below
            # consumes 512-seq-col blocks, so the PE starts after wk + the
            # FIRST 1MB of x (~5us) instead of the whole 4MB (~24us).
            xT_t = xw.tile([128, 4, KT, 512], BF16, tag="xT")
            wts = {}
            for nm in ("wk", "wq", "wv"):
                wts[nm] = xw.tile([128, KT, HD], BF16, tag=nm, name=nm)
            xr = d["xT_d"].ap()
            nc.sync.dma_start(out=wts["wk"], in_=d["wkT_d"].ap())
            nc.sync.dma_start(out=xT_t[:, 0], in_=xr[:, 0])
            nc.sync.dma_start(out=xT_t[:, 1], in_=xr[:, 1])
            nc.sync.dma_start(out=wts["wq"], in_=d["wqT_d"].ap())
            nc.sync.dma_start(out=xT_t[:, 2], in_=xr[:, 2])
            nc.sync.dma_start(out=xT_t[:, 3], in_=xr[:, 3])
            nc.sync.dma_start(out=wts["wv"], in_=d["wvT_d"].ap())

            knat = [qkv2.tile([128, NCH, 128], BF16, tag=f"knat{m}",
                              name=f"knat{m}") for m in range(2)]
            kvs = d["kv_scratch"]
            # block-diagonal q for the pair-merged G matmuls: built by the
            # otherwise-idle Pool engine from the q epilogues (off-blocks
            # zeroed up front, also on Pool).
            qblk = [qkv2.tile([128, NCH, 256], BF16, tag=f"qblk{m}",
                              name=f"qblk{m}") for m in range(2)]
            for m in range(2):
                nc.gpsimd.memset(qblk[m][:, :, :], 0.0)

            # n-OUTER waves: psum (n, m) accumulates its 8 kt matmuls
            # back-to-back and needs only x s-block n, so wave 0 runs as
            # soon as the first 1MB of x lands and later waves chase the
            # DMA stream (1MB every ~2.8us vs 3.4us of PE work per block).
            epi = 0

            def qk_waves(nm, dst, bias, blkdst=None):
                nonlocal epi
                for n in range(4):
                    for m in range(2):
                        ps = ppj.tile([128, 512], F32, tag="pj",
                                      name=f"pj{n}{m}")
                        for kt in range(KT):
                            nc.tensor.matmul(
                                ps, wts[nm][:, kt, 128 * m:128 * m + 128],
                                xT_t[:, n, kt, :],
                                start=(kt == 0), stop=(kt == KT - 1))
                        sl = slice(512 * n, 512 * n + 512)
                        if epi % 2 == 0:
                            nc.vector.tensor_scalar(
                                out=dst[m][:, sl], in0=ps,
                                scalar1=bias[:, m:m + 1], scalar2=None,
                                op0=mybir.AluOpType.add)
                        else:
                            nc.scalar.activation(
                                out=dst[m][:, sl], in_=ps,
                                func=IDENT, bias=bias[:, m:m + 1])
                        if blkdst is not None:
                            src = dst[m][:, sl].rearrange(
                                "p (i t) -> p i t", i=4)
                            nc.gpsimd.tensor_copy(
                                out=blkdst[m][0:64, 4 * n:4 * n + 4, 0:128],
                                in_=src[0:64])
                            nc.gpsimd.tensor_copy(
                                out=blkdst[m][64:128, 4 * n:4 * n + 4,
                                              128:256],
                                in_=src[64:128])
                        epi += 1

            # k first, then q, then v
            qk_waves("wk", kT, bk_t)
            # kick the k natural-layout DRAM round-trip early so the
            # xbar transpose lands while v projects
            for m in range(2):
                nc.sync.dma_start(out=kvs[m].ap(), in_=kT[m][:, :])
                nc.sync.dma_start_transpose(out=knat[m], in_=kvs[m].ap())
            qk_waves("wq", qT, bq_t, blkdst=qblk)

            # v directly in natural (s, hd) layout: out = xT.T @ wvT. No
            # bias matmul: softmax weights sum to 1, so +bv passes straight
            # through both attention levels; the host folds Wo@bv into bo.
            for wv in range(4):
                vps = [ppj.tile([128, HD], F32, tag="vp", name=f"vp{j}")
                       for j in range(4)]
                for kt in range(KT):
                    for j in range(4):
                        sc = 4 * wv + j
                        nc.tensor.matmul(
                            vps[j],
                            xT_t[:, sc // 4, kt,
                                 T * (sc % 4):T * (sc % 4) + T],
                            wts["wv"][:, kt, :],
                            start=(kt == 0), stop=(kt == KT - 1))
                for j in range(4):
                    sc = 4 * wv + j
                    if sc % 2 == 0:
                        nc.vector.tensor_copy(out=vnat[:, sc, :], in_=vps[j])
                    else:
                        nc.scalar.copy(out=vnat[:, sc, :], in_=vps[j])

        # ---------- w16, packed wT, den carries, SK/SV prefixes ----------
        w16 = wstore.tile([C, HPC, S], BF16, tag="w16")
        # wT packed (t, 32h+c): cols 16-31 of each 32-group hold exp(0)=1
        # from the zero-padded qc -- written, never read back.
        wT_pad = wstore.tile([128, NCH, HPC, 32], BF16, tag="wTp")
        carry_cols = wstore.tile([128, NCH], F32, tag="carryc")
        # SK prefix state, BLOCK-DIAGONAL per head pair: partitions (j, d)
        # stack heads 2m/2m+1 exactly like kT[m]/qT[m], cols (j', c) hold
        # head 2m+j' only on the diagonal j'==j (off-blocks zeroed once by
        # the idle Pool engine). stage_a then needs only TWO SK^T q matmuls
        # per chunk -- qT[m] is already the pair-stacked moving operand.
        sk_bd = [wstore.tile([128, NCH, 64], BF16, tag=f"skbd{m}",
                             name=f"skbd{m}")
                 for m in range(2)]
        for m in range(2):
            nc.gpsimd.memset(sk_bd[m][:, :, :], 0.0)
        sk_all = wstore.tile([64, NCH, HPC, 32], BF16, tag="skal")
        sv_all = wstore.tile([C, NCH, HPC, DH], BF16, tag="sv")

        w16p = gstore.tile([128, S], BF16, tag="w16p")
        with tc.tile_pool(name="ph3a", bufs=3, space="PSUM") as ph3a:
            # w16 = exp(qc . k): the block-diag qcb emits BOTH heads of a
            # pair per matmul (kT[m] stacks them on 128 partitions), ONE exp
            # per 512-slice, then 4 sbuf-to-sbuf DMAs remap rows to the
            # (c, h, s) layout.
            for n in range(4):
                ps = ph3a.tile([128, 512], F32, tag="w16ps")
                for m in range(2):
                    nc.tensor.matmul(
                        ps[64 * m:64 * m + 64, :], qcb_t[:, m, :],
                        kT[m][:, 512 * n:512 * n + 512],
                        start=True, stop=True, tile_position=(0, 64 * m))
                nc.scalar.activation(
                    out=w16p[:, 512 * n:512 * n + 512], in_=ps, func=EXP)
            for h in range(HPC):
                nc.sync.dma_start(out=w16[:, h, :],
                                  in_=w16p[32 * h:32 * h + C, :])

        with tc.tile_pool(name="ph3b", bufs=4, space="PSUM") as ph3b:
            # wT (t, 32h+c) = exp(k . qc_pad): 2 matmuls/chunk via the
            # stacked-pair trick, exp batched over chunk PAIRS so the Act
            # queue doesn't trail the PE. (The den carry chain now rides on
            # the den psum's last column inside stage_a -- no cs matmuls.)
            for i2 in range(NCH // 2):
                tp = ph3b.tile([128, 2, HPC, 32], F32, tag="tp")
                for di in range(2):
                    i = 2 * i2 + di
                    ch = slice(T * i, T * i + T)
                    for m in range(2):
                        nc.tensor.matmul(
                            tp[:, di, 2 * m:2 * m + 2, :], kT[m][:, ch],
                            qcb_t[:, m, :], start=True, stop=True)
                nc.scalar.activation(
                    out=wT_pad[:, 2 * i2:2 * i2 + 2], in_=tp, func=EXP)

        # The loop's psum pools open HERE, before G/ph3c, and those phases
        # draw from the same tag rings (same shapes): a tile_pool open/close
        # emits Pool-engine marker instructions, and any pool transition
        # between ph3c and the loop would serialize the loop start behind
        # the Pool engine's sk_bd copy tail.
        pda = ep(tc.tile_pool(name="pda", bufs=2, space="PSUM"))
        pbig = ep(tc.tile_pool(name="pbig", bufs=3, space="PSUM"))  # pw+mr
        pop = ep(tc.tile_pool(name="pop", bufs=3, space="PSUM"))

        # ---------- G = (k^T q) o U for all chunks (keeps PE dense) ----------
        gmt_all = gstore.tile([128, NCH, 4 * T], BF16, tag="gmt")
        for i in range(NCH):
            ch = slice(T * i, T * i + T)
            gt = pbig.tile([128, 512], F32, tag="big", name=f"gt{i}")
            for m in range(2):
                nc.tensor.matmul(
                    gt[:, 256 * m:256 * m + 256],
                    kT[m][:, ch], qblk[m][:, i, :], start=True, stop=True)
            nc.vector.tensor_mul(gmt_all[:, i, :], gt, u4_t)

        if True:
            # SK/SV chunk deltas + exclusive prefixes (slot i = state
            # before chunk i; slot 0 unused). Deltas stay per-head: the
            # prefix chain then needs only TWO wide DVE adds per boundary
            # (DVE is the scarce engine here -- only it can read PSUM for
            # tensor+tensor).
            for i in range(NCH - 1):
                ksv = pop.tile([128, 512], F32, tag="ops", name=f"ksv{i}")
                skd = ksv[0:64, 0:128].rearrange("p (h c) -> p h c", h=HPC)
                svd = ksv[0:C, 128:384].rearrange("p (h e) -> p h e", h=HPC)
                for h in range(HPC):
                    nc.tensor.matmul(
                        skd[:, h, :],
                        knat[h // 2][:, i, 64 * (h % 2):64 * (h % 2) + 64],
                        wT_pad[:, i, h, :], start=True, stop=True)
                    nc.tensor.matmul(
                        svd[:, h, :], wT_pad[:, i, h, 0:C],
                        vnat[:, i, 64 * h:64 * h + 64], start=True, stop=True)
                # prefix adds stay WIDE and packed on DVE (2 ops, short
                # serial chain); the idle Pool fans sk out to the
                # block-diagonal layout OFF the chain.
                if i == 0:
                    nc.scalar.copy(out=sk_all[:, 1], in_=skd)
                    nc.scalar.copy(out=sv_all[:, 1], in_=svd)
                else:
                    nc.vector.tensor_add(sk_all[:, i + 1], skd, sk_all[:, i])
                    nc.vector.tensor_add(sv_all[:, i + 1], svd, sv_all[:, i])
                # fan-out on the idle Pool engine, off the DVE prefix chain
                for h in range(HPC):
                    m, j = h // 2, h % 2
                    nc.gpsimd.tensor_copy(
                        out=sk_bd[m][64 * j:64 * j + 64, i + 1,
                                     32 * j:32 * j + 32],
                        in_=sk_all[:, i + 1, h, :])

        # ---------- main chunk loop, software-pipelined ----------
        # Per-engine sequencers run IN ORDER, so a stage whose inputs are
        # still in flight head-of-line-blocks everything behind it. Stage
        # the emission: C(i-2) out-proj, B(i-1) PW/mix, A(i) den/att --
        # each stage's inputs were produced >= 1 iteration earlier.
        o_nm = onum.tile([128, 2, S], BF16, tag="o_nm")
        rb_all = onum.tile([128, 2, S], BF16, tag="rb_all")

        ctiles = {}

        def stage_a(i):
            ch = slice(T * i, T * i + T)
            # den (packed 32h+c) in ONE matmul; att numerator per head into
            # the same packed layout via tile_position column offsets. For
            # i>0 the SK_i^T q_i state term accumulates into the same psum
            # region (sk_all is precomputed, so no chunk->chunk dependency).
            da = pda.tile([128, 2, T], F32, tag="da")
            den_b = da[:, 0, :]
            an_b = da[:, 1, :]
            nc.tensor.matmul(den_b, wT_pad[:, i], u128, start=True, stop=True)
            for h in range(HPC):
                nc.tensor.matmul(
                    an_b[32 * h:32 * h + 32, :], wT_pad[:, i, h, :],
                    gmt_all[:, i, 128 * h:128 * h + 128],
                    start=True, stop=(i == 0), tile_position=(0, 32 * h))
            if i > 0:
                # block-diag SK state: one matmul per head PAIR (qT[m] is
                # the stacked moving operand), 64 packed rows at a time.
                for m in range(2):
                    nc.tensor.matmul(
                        an_b[64 * m:64 * m + 64, :], sk_bd[m][:, i, :],
                        qT[m][:, ch], start=False, stop=True,
                        tile_position=(0, 64 * m), skip_group_check=True)

            # den carry chain rides on the den psum's last column (the
            # intra-chunk cumsum's final entry IS the chunk total).
            if i < NCH - 1:
                if i == 0:
                    nc.scalar.copy(out=carry_cols[:, 1:2],
                                   in_=den_b[:, T - 1:T])
                else:
                    nc.vector.tensor_add(carry_cols[:, i + 1:i + 2],
                                         den_b[:, T - 1:T],
                                         carry_cols[:, i:i + 1])

            # softmax pieces, packed layout: 1/x as exp(-ln x), den carry
            # folded into the ln bias (per-partition column).
            lden = sb2.tile([128, T], F32, tag="lden")
            if i == 0:
                nc.scalar.activation(out=lden, in_=den_b, func=LN)
            else:
                nc.scalar.activation(out=lden, in_=den_b, func=LN,
                                     bias=carry_cols[:, i:i + 1])
            rden = sb2.tile([128, T], F32, tag="rden")
            nc.scalar.activation(out=rden, in_=lden, func=EXP, scale=-1.0)
            att = sb2.tile([128, T], F32, tag="att")
            nc.vector.tensor_mul(att, an_b, rden)
            e_b = sb2.tile([128, T], BF16, tag="e")
            nc.scalar.activation(out=e_b, in_=att, func=EXP)
            # pdd directly in the (c, h, t) matmul layout: 4 small Pool
            # muls on the real c-rows -- no packed intermediate and no DMA
            # remaps (the SP sequencer's 565ns/issue was loop-critical).
            pdd_u = sb2.tile([C, HPC, T], BF16, tag="pddu")
            for h in range(HPC):
                nc.gpsimd.tensor_mul(pdd_u[:, h, :],
                                     e_b[32 * h:32 * h + C, :],
                                     rden[32 * h:32 * h + C, :])
            ctiles[i] = (e_b, pdd_u)

        obi = [0]

        def oproj_block(c0, nch=4):
            """Normalize + out-project + store chunks [c0, c0+nch). Emitted
            >= 3 chunks behind the producers, so this is a dense matmul
            burst -- long-ready inputs, no head-of-line blocking."""
            bs = slice(T * c0, T * c0 + nch * T)
            oT_b = otp.tile([128, 2, 4 * T], BF16, tag=f"oTb{obi[0] % 2}")
            obi[0] += 1
            oT_b = oT_b[:, :, 0:nch * T]
            # all-SBUF bf16 multiply -> GpSimd (otherwise idle; DVE is loaded)
            nc.gpsimd.tensor_mul(oT_b, o_nm[:, :, bs], rb_all[:, :, bs])
            for sc in range(nch):
                ob = outp.tile([128, D], BF16, tag="ob")
                for n2 in range(2):
                    ps = pop.tile([128, 512], F32, tag="ops")
                    for kt in range(2):
                        nc.tensor.matmul(
                            ps, oT_b[:, kt, T * sc:T * sc + T],
                            woT_t[:, kt, 512 * n2:512 * n2 + 512],
                            start=(kt == 0), stop=(kt == 1))
                    if n2 == 0:
                        nc.vector.tensor_copy(
                            out=ob[:, 512 * n2:512 * n2 + 512], in_=ps)
                    else:
                        nc.scalar.copy(
                            out=ob[:, 512 * n2:512 * n2 + 512], in_=ps)
                nc.sync.dma_start(
                    out=d["out_d"].ap()[T * (c0 + sc):T * (c0 + sc) + T, :],
                    in_=ob)

        def stage_b(i):
            ch = slice(T * i, T * i + T)
            e_b, pdd_u = ctiles[i]
            # PW (t, s) + mask
            pw = pbig.tile([128, 512], F32, tag="big")
            for h in range(HPC):
                nc.tensor.matmul(
                    pw[:, 128 * h:128 * h + 128], w16[:, h, ch],
                    pdd_u[:, h, :], start=True, stop=True)
            pwm = sb2.tile([128, 512], BF16, tag="pwm")
            nc.vector.tensor_mul(pwm, pw, u4_t)

            # o numerator (cols 0-255) + sums broadcast (cols 256-511).
            # rb and sv matmuls first: their inputs (e_b, pdd_u) landed an
            # iteration ago, so they bridge the ~790ns the PE would
            # otherwise idle waiting for the pwm mask-mul on DVE.
            mr = pbig.tile([128, 512], F32, tag="big")
            for pair in range(2):
                nc.tensor.matmul(
                    mr[:, 256 + 128 * pair:384 + 128 * pair],
                    sel2_t[:, pair, :], e_b, start=True, stop=True)
            for h in (0, 2, 1, 3):          # bp-stable order
                bp = 64 * (h % 2)
                cb = 128 * (h // 2)
                nc.tensor.matmul(
                    mr[bp:bp + 64, cb:cb + 128],
                    vnat[:, i, 64 * h:64 * h + 64],
                    pwm[:, 128 * h:128 * h + 128],
                    start=True, stop=(i == 0), tile_position=(0, bp))
                if i > 0:
                    nc.tensor.matmul(
                        mr[bp:bp + 64, cb:cb + 128],
                        sv_all[:, i, h, :], pdd_u[:, h, :],
                        start=False, stop=True, tile_position=(0, bp))
            nc.vector.tensor_copy(out=o_nm[:, :, ch], in_=mr[:, 0:256])
            lsb = sb2.tile([128, 256], F32, tag="lsb")
            nc.scalar.activation(out=lsb, in_=mr[:, 256:512], func=LN)
            nc.scalar.activation(out=rb_all[:, :, ch], in_=lsb,
                                 func=EXP, scale=-1.0)

        for i in range(NCH + 2):
            if 2 <= i:
                stage_b(i - 2)          # lag 2: A(i)'s chain has ~2
            if i < NCH:                 # iterations to land before B reads it
                stage_a(i)
            if i == 10:
                oproj_block(0)          # 3+ chunks behind the producers
            elif i == 13:
                oproj_block(4)
            elif i == 16:
                oproj_block(8)
            elif i == 17:
                oproj_block(12, 2)      # small tail blocks: the final
        oproj_block(14, 2)              # post-loop burst is ~half as long


def kernel(**inputs):
    global LAST_EXEC_NS
    x = np.ascontiguousarray(inputs["x"], np.float32)
    q_c, beta = np.asarray(inputs["q_c"]), np.asarray(inputs["beta"])
    Wq, bq = np.asarray(inputs["Wq"]), np.asarray(inputs["bq"])
    Wk, bk = np.asarray(inputs["Wk"]), np.asarray(inputs["bk"])
    Wv, bv = np.asarray(inputs["Wv"]), np.asarray(inputs["bv"])
    Wo, bo = np.asarray(inputs["Wo"]), np.asarray(inputs["bo"])

    if "nc" not in _CACHE:
        _CACHE["nc"] = _build()
    nc = _CACHE["nc"]

    BF = ml_dtypes.bfloat16
    # per-head query temperature 0.125*exp(-beta) folded into Wq/bq
    qscale = (0.125 * np.exp(-beta.astype(np.float64))).astype(np.float32)
    qs_hd = np.repeat(qscale, DH)                      # (D,) per out-dim
    Wq_s = Wq * qs_hd[:, None]
    bq_s = bq * qs_hd

    in_maps = []
    for core in range(8):
        b, hh = core // 4, core % 4
        hd = slice(hh * HD, hh * HD + HD)
        # qc block-diagonal per head pair, matching kT[m]'s [h_2m; h_2m+1]
        # partition stacking; zero c-pad 16->32.
        qcb = np.zeros((2, 128, 64), BF)
        qc_r = q_c[:, hd].reshape(C, HPC, DH)          # (c, h, d)
        for m in range(2):
            qcb[m, 0:64, 0:C] = qc_r[:, 2 * m, :].T.astype(BF)
            qcb[m, 64:128, 32:32 + C] = qc_r[:, 2 * m + 1, :].T.astype(BF)
        # pre-shuffles: one contiguous DRAM run per SBUF partition line.
        # xT (D, S) -> (p, nblk, kt, s'); w*T (D, HD) -> (p, kt, j);
        # woT (HD, D) -> (p, kt2, j).
        def wshuf(w):          # (D, HD) -> (128, KT, HD)
            return np.ascontiguousarray(
                w.reshape(KT, 128, HD).transpose(1, 0, 2).astype(BF))
        xT = x[:, b, :].T.astype(BF)                    # (D, S)
        xsh = np.ascontiguousarray(
            xT.reshape(KT, 128, 4, 512).transpose(1, 2, 0, 3))
        in_maps.append({
            "xT": xsh,
            "wqT": wshuf(Wq_s[hd, :].T),
            "wkT": wshuf(Wk[hd, :].T),
            "wvT": wshuf(Wv[hd, :].T),
            "woT": np.ascontiguousarray(
                Wo[:, hd].T.reshape(2, 128, D).transpose(1, 0, 2).astype(BF)),
            "qcb": qcb,
            "bq": np.ascontiguousarray(bq_s[hd].reshape(2, 128).T),
            "bk": np.ascontiguousarray(bk[hd].reshape(2, 128).T),
        })

    trace = os.environ.get("TRN_PROFILE", "0") == "1"
    res = run_bass_kernel_spmd(nc, in_maps, list(range(8)), trace=trace)
    LAST_EXEC_NS = res.exec_time_ns

    out = np.zeros((S, B, D), np.float32)
    for core in range(8):
        out[:, core // 4, :] += res.results[core]["out_p"].astype(np.float32)
    # v-bias folded out of the kernel: softmax weights sum to 1, so +bv
    # passes through both attention levels and lands as a constant Wo@bv.
    out += (bo + Wo @ bv.astype(np.float32))[None, None, :]
    return out



# revision 62
# speedup vs baseline: 1.0061x; 1.0061x over previous
"""CompressionAttention Trainium2 kernel (8 NeuronCores, SPMD).

Sharding: core i handles batch b=i//4 and 4 heads hh=i%4 (model-dim slice
hh*256:(hh+1)*256). Heads never interact before out_proj, so each core
computes a partial out-projection for its batch; the host gather sums the
4 partials per batch and adds bo once.

Algorithm per core (chunked linear attention, chunk T=128):
  w[c,t] = exp(qc_c . k_t)            (max-subtraction dropped: att is
                                       invariant to per-c scaling of w)
  den[c,s]   = cumsum_t<=s w[c,t]
  att[c,s]   = (w_chunk^T (U o G) + SK^T qs) / den   per chunk + state
  p = softmax_c att ; o_s = sum_c p * (sum_t<=s w v)/den

Schedule: all cross-chunk recurrences (den carry, SK/SV prefixes) and all
G = k^T q blocks are precomputed in side phases, so the main chunk loop has
no chunk->chunk dependency and the Tensor engine never idles long enough to
HAM-throttle. Softmax elementwise ops run in a partition-packed (32h+c, t)
layout (engine time ~ free-size, so 4x cheaper than (c, 4h*t)); the packed
e/pdd are re-laid-out for their matmul consumers with tiny SBUF-to-SBUF
DMAs. 1/x runs as exp(-ln(x)) on Act (single act table, den carry folded
into the ln bias); 1/sums o-normalization is deferred out of the loop.
"""
import os
import numpy as np
import ml_dtypes

import concourse.bacc as bacc
import concourse.tile as tile
from concourse import mybir
from concourse.bass_utils import run_bass_kernel_spmd

F32 = mybir.dt.float32
BF16 = mybir.dt.bfloat16
EXP = mybir.ActivationFunctionType.Exp
LN = mybir.ActivationFunctionType.Ln
IDENT = mybir.ActivationFunctionType.Identity

S, B, D = 2048, 2, 1024
H, DH, C = 16, 64, 16
HPC = 4            # heads per core
HD = HPC * DH      # 256 model dims per core
T = 128            # chunk
NCH = S // T       # 16 chunks
KT = D // 128      # 8 k-tiles of the model dim

LAST_EXEC_NS = None
_CACHE = {}


def _patched_insert_act_table_loads(self):
    """Force every activation (Exp/Ln/Copy/Identity) onto the single
    act-func table that contains them all. The default greedy pass picks
    exp_and_others for Exp and natural_log_exp_and_others for Ln, inserting
    an ACT_TABLE_LOAD (1.28us!) on every switch."""
    import bass_rust as _bass_rust
    from concourse.hw_specs import get_activation_tables
    has_activation = any(
        isinstance(i, mybir.InstActivation)
        for b in self.main_func.blocks
        for i in b.instructions)
    if not has_activation:
        return
    keep = {mybir.ActivationFunctionType.Exp, mybir.ActivationFunctionType.Ln,
            mybir.ActivationFunctionType.Copy,
            mybir.ActivationFunctionType.Identity}
    tables = []
    for name, funcs in get_activation_tables(self.m.arch).items():
        if name != "natural_log_exp_and_others":
            funcs = funcs - keep
        tables.append((name, funcs))
    _bass_rust.insert_act_table_loads(self, tables)


def _build():
    nc = bacc.Bacc("TRN2", target_bir_lowering=False, debug=False, num_devices=8)
    import types as _types
    nc.insert_act_table_loads = _types.MethodType(
        _patched_insert_act_table_loads, nc)

    # ---- DRAM I/O (per-core, host-prepped layouts; bf16 where possible).
    # x and weights are pre-shuffled on host so each SBUF partition line is
    # ONE contiguous DRAM run (8KB / 4KB descriptors -> near-peak DMA bw;
    # the natural (kt p) layouts gave 512B-1KB descriptors, ~3x slower). ----
    xT_d = nc.dram_tensor("xT", [128, 4, KT, 512], BF16, kind="ExternalInput")
    wqT_d = nc.dram_tensor("wqT", [128, KT, HD], BF16, kind="ExternalInput")
    wkT_d = nc.dram_tensor("wkT", [128, KT, HD], BF16, kind="ExternalInput")
    wvT_d = nc.dram_tensor("wvT", [128, KT, HD], BF16, kind="ExternalInput")
    woT_d = nc.dram_tensor("woT", [128, 2, D], BF16, kind="ExternalInput")
    # qc block-diagonal per head PAIR: rows 0-63 = head 2m's d, rows 64-127 =
    # head 2m+1's d (matching the [k_2m; k_2m+1] stacking of kT[m]), cols =
    # (j, c) with zero off-blocks and zero c-pad 16->32 (pad logits are 0 ->
    # w=1, never read). One matmul then emits BOTH heads' wT/w16 block.
    qcb_d = nc.dram_tensor("qcb", [2, 128, 64], BF16, kind="ExternalInput")
    # biases host-prepped as (p, m) so each partition line is one contiguous
    # 8B descriptor (the (m p)->p m rearrange of a flat [256] makes 256
    # 4-byte descriptors).
    bq_d = nc.dram_tensor("bq", [128, 2], F32, kind="ExternalInput")
    bk_d = nc.dram_tensor("bk", [128, 2], F32, kind="ExternalInput")
    out_d = nc.dram_tensor("out_p", [S, D], BF16, kind="ExternalOutput")

    # ---- consts baked into the NEFF ----
    u = np.triu(np.ones((T, T), np.float32))
    u4 = np.tile(u, (1, 4)).astype(ml_dtypes.bfloat16)
    u4_d = nc.inline_tensor(u4, "u4c")
    # sel2[pair][p, j] = 1 iff p selects head 2*pair (j<64) / 2*pair+1
    # (j>=64): the sums-broadcast matmul consumes packed (32h+c) e directly
    # and emits both heads of a column-block in one matmul.
    sel2 = np.zeros((2, 128, 128), np.float32)
    for pair in range(2):
        sel2[pair, 64 * pair:64 * pair + C, 0:64] = 1.0
        sel2[pair, 64 * pair + 32:64 * pair + 32 + C, 64:128] = 1.0
    sel2_d = nc.inline_tensor(
        np.ascontiguousarray(sel2.transpose(1, 0, 2)).astype(ml_dtypes.bfloat16),
        "sel2")
    kv_scratch = [nc.dram_tensor(f"kvs{j}", [128, 2048], BF16) for j in range(2)]

    with tile.TileContext(nc) as tc:
        _emit(nc, tc, locals())
    nc.compile()
    return nc


def _emit(nc, tc, d):
    from contextlib import ExitStack

    with ExitStack() as ctx:
        ep = ctx.enter_context

        # ---------- persistent pools ----------
        consts = ep(tc.tile_pool(name="consts", bufs=1))
        wpool = ep(tc.tile_pool(name="wpool", bufs=1))      # WoT
        qkv = ep(tc.tile_pool(name="qkv", bufs=1))          # qT/kT/vnat
        qkv2 = ep(tc.tile_pool(name="qkv2", bufs=1))        # knat, row-0 copies
        wstore = ep(tc.tile_pool(name="wstore", bufs=1))    # w16/wT/carry/sk/sv
        gstore = ep(tc.tile_pool(name="gstore", bufs=1))    # masked G, all chunks
        onum = ep(tc.tile_pool(name="onum", bufs=1))        # o_num, rb_all
        sb2 = ep(tc.tile_pool(name="sb2", bufs=3))          # per-chunk sbuf
        otp = ep(tc.tile_pool(name="otp", bufs=1))          # oT final
        outp = ep(tc.tile_pool(name="outp", bufs=3))        # out staging

        # ---------- consts (issued on the Act HWDGE sequencer so the SP
        # sequencer's 565ns/issue budget is spent only on the big x/weight
        # loads; Act is idle during the load phase anyway) ----------
        def bias_tile(name):
            t = consts.tile([128, 2], F32, tag=name)
            nc.scalar.dma_start(out=t, in_=d[name + "_d"].ap())
            return t
        bk_t, bq_t = bias_tile("bk"), bias_tile("bq")
        u4_t = consts.tile([128, 4 * T], BF16, tag="u4")
        nc.scalar.dma_start(out=u4_t, in_=d["u4_d"].ap())
        u128 = u4_t[:, 0:T]
        sel2_t = consts.tile([128, 2, 128], BF16, tag="sel2")
        nc.scalar.dma_start(out=sel2_t, in_=d["sel2_d"].ap())
        qcb_t = consts.tile([128, 2, 64], BF16, tag="qcb")
        nc.scalar.dma_start(
            out=qcb_t, in_=d["qcb_d"].ap().rearrange("m p j -> p m j"))

        # WoT resident (bf16): (128, 2, D); only needed at the first oproj
        # block mid-loop, so it loads on Act after the consts.
        woT_t = wpool.tile([128, 2, D], BF16, tag="woT")
        nc.scalar.dma_start(out=woT_t, in_=d["woT_d"].ap())

        # ---------- projections (all bf16) ----------
        qT = [qkv.tile([128, S], BF16, tag=f"qT{m}", name=f"qT{m}") for m in range(2)]
        kT = [qkv.tile([128, S], BF16, tag=f"kT{m}", name=f"kT{m}") for m in range(2)]
        vnat = qkv.tile([128, NCH, HD], BF16, tag="vnat", name="vnat")

        with tc.tile_pool(name="xw", bufs=1) as xw, \
                tc.tile_pool(name="ppj", bufs=4, space="PSUM") as ppj:
            # DMA order = consumption order, ONE dma_start per logical block
            # (descriptors spray across all 16 DMA engines, so a block moves
            # at aggregate bandwidth; splitting it only costs SP issue slots
            # at 565ns each). x goes s-block-major: the n-outer wave below
            # consumes 512-seq-col blocks, so the PE starts after wk + the
            # FIRST 1MB of x (~5us) instead of the whole 4MB (~24us).
            xT_t = xw.tile([128, 4, KT, 512], BF16, tag="xT")
            wts = {}
            for nm in ("wk", "wq", "wv"):
                wts[nm] = xw.tile([128, KT, HD], BF16, tag=nm, name=nm)
            xr = d["xT_d"].ap()
            nc.sync.dma_start(out=wts["wk"], in_=d["wkT_d"].ap())
            nc.sync.dma_start(out=xT_t[:, 0], in_=xr[:, 0])
            nc.sync.dma_start(out=xT_t[:, 1], in_=xr[:, 1])
            nc.sync.dma_start(out=wts["wq"], in_=d["wqT_d"].ap())
            nc.sync.dma_start(out=xT_t[:, 2], in_=xr[:, 2])
            nc.sync.dma_start(out=xT_t[:, 3], in_=xr[:, 3])
            nc.sync.dma_start(out=wts["wv"], in_=d["wvT_d"].ap())

            knat = [qkv2.tile([128, NCH, 128], BF16, tag=f"knat{m}",
                              name=f"knat{m}") for m in range(2)]
            kvs = d["kv_scratch"]
            # block-diagonal q for the pair-merged G matmuls: built by the
            # otherwise-idle Pool engine from the q epilogues (off-blocks
            # zeroed up front, also on Pool).
            qblk = [qkv2.tile([128, NCH, 256], BF16, tag=f"qblk{m}",
                              name=f"qblk{m}") for m in range(2)]
            for m in range(2):
                nc.gpsimd.memset(qblk[m][:, :, :], 0.0)

            # n-OUTER waves: psum (n, m) accumulates its 8 kt matmuls
            # back-to-back and needs only x s-block n, so wave 0 runs as
            # soon as the first 1MB of x lands and later waves chase the
            # DMA stream (1MB every ~2.8us vs 3.4us of PE work per block).
            epi = 0

            def qk_waves(nm, dst, bias, blkdst=None):
                nonlocal epi
                for n in range(4):
                    for m in range(2):
                        ps = ppj.tile([128, 512], F32, tag="pj",
                                      name=f"pj{n}{m}")
                        for kt in range(KT):
                            nc.tensor.matmul(
                                ps, wts[nm][:, kt, 128 * m:128 * m + 128],
                                xT_t[:, n, kt, :],
                                start=(kt == 0), stop=(kt == KT - 1))
                        sl = slice(512 * n, 512 * n + 512)
                        if epi % 2 == 0:
                            nc.vector.tensor_scalar(
                                out=dst[m][:, sl], in0=ps,
                                scalar1=bias[:, m:m + 1], scalar2=None,
                                op0=mybir.AluOpType.add)
                        else:
                            nc.scalar.activation(
                                out=dst[m][:, sl], in_=ps,
                                func=IDENT, bias=bias[:, m:m + 1])
                        if blkdst is not None:
                            src = dst[m][:, sl].rearrange(
                                "p (i t) -> p i t", i=4)
                            nc.gpsimd.tensor_copy(
                                out=blkdst[m][0:64, 4 * n:4 * n + 4, 0:128],
                                in_=src[0:64])
                            nc.gpsimd.tensor_copy(
                                out=blkdst[m][64:128, 4 * n:4 * n + 4,
                                              128:256],
                                in_=src[64:128])
                        epi += 1

            # k first, then q, then v
            qk_waves("wk", kT, bk_t)
            # kick the k natural-layout DRAM round-trip early so the
            # xbar transpose lands while v projects
            for m in range(2):
                nc.sync.dma_start(out=kvs[m].ap(), in_=kT[m][:, :])
                nc.sync.dma_start_transpose(out=knat[m], in_=kvs[m].ap())
            qk_waves("wq", qT, bq_t, blkdst=qblk)

            # v directly in natural (s, hd) layout: out = xT.T @ wvT. No
            # bias matmul: softmax weights sum to 1, so +bv passes straight
            # through both attention levels; the host folds Wo@bv into bo.
            for wv in range(4):
                vps = [ppj.tile([128, HD], F32, tag="vp", name=f"vp{j}")
                       for j in range(4)]
                for kt in range(KT):
                    for j in range(4):
                        sc = 4 * wv + j
                        nc.tensor.matmul(
                            vps[j],
                            xT_t[:, sc // 4, kt,
                                 T * (sc % 4):T * (sc % 4) + T],
                            wts["wv"][:, kt, :],
                            start=(kt == 0), stop=(kt == KT - 1))
                for j in range(4):
                    sc = 4 * wv + j
                    if sc % 2 == 0:
                        nc.vector.tensor_copy(out=vnat[:, sc, :], in_=vps[j])
                    else:
                        nc.scalar.copy(out=vnat[:, sc, :], in_=vps[j])

        # ---------- w16, packed wT, den carries, SK/SV prefixes ----------
        w16 = wstore.tile([C, HPC, S], BF16, tag="w16")
        # wT packed (t, 32h+c): cols 16-31 of each 32-group hold exp(0)=1
        # from the zero-padded qc -- written, never read back.
        wT_pad = wstore.tile([128, NCH, HPC, 32], BF16, tag="wTp")
        carry_cols = wstore.tile([128, NCH], F32, tag="carryc")
        # SK prefix state, BLOCK-DIAGONAL per head pair: partitions (j, d)
        # stack heads 2m/2m+1 exactly like kT[m]/qT[m], cols (j', c) hold
        # head 2m+j' only on the diagonal j'==j (off-blocks zeroed once by
        # the idle Pool engine). stage_a then needs only TWO SK^T q matmuls
        # per chunk -- qT[m] is already the pair-stacked moving operand.
        sk_bd = [wstore.tile([128, NCH, 64], BF16, tag=f"skbd{m}",
                             name=f"skbd{m}")
                 for m in range(2)]
        for m in range(2):
            nc.gpsimd.memset(sk_bd[m][:, :, :], 0.0)
        sk_all = wstore.tile([64, NCH, HPC, 32], BF16, tag="skal")
        sv_all = wstore.tile([C, NCH, HPC, DH], BF16, tag="sv")

        def skbd_fanout(s):
            for h in range(HPC):
                m, j = h // 2, h % 2
                nc.gpsimd.tensor_copy(
                    out=sk_bd[m][64 * j:64 * j + 64, s, 32 * j:32 * j + 32],
                    in_=sk_all[:, s, h, :])

        w16p = gstore.tile([128, S], BF16, tag="w16p")
        with tc.tile_pool(name="ph3a", bufs=3, space="PSUM") as ph3a:
            # w16 = exp(qc . k): the block-diag qcb emits BOTH heads of a
            # pair per matmul (kT[m] stacks them on 128 partitions), ONE exp
            # per 512-slice, then 4 sbuf-to-sbuf DMAs remap rows to the
            # (c, h, s) layout.
            for n in range(4):
                ps = ph3a.tile([128, 512], F32, tag="w16ps")
                for m in range(2):
                    nc.tensor.matmul(
                        ps[64 * m:64 * m + 64, :], qcb_t[:, m, :],
                        kT[m][:, 512 * n:512 * n + 512],
                        start=True, stop=True, tile_position=(0, 64 * m))
                nc.scalar.activation(
                    out=w16p[:, 512 * n:512 * n + 512], in_=ps, func=EXP)
            for h in range(HPC):
                nc.sync.dma_start(out=w16[:, h, :],
                                  in_=w16p[32 * h:32 * h + C, :])

        with tc.tile_pool(name="ph3b", bufs=4, space="PSUM") as ph3b:
            # wT (t, 32h+c) = exp(k . qc_pad): 2 matmuls/chunk via the
            # stacked-pair trick, exp batched over chunk PAIRS so the Act
            # queue doesn't trail the PE. (The den carry chain now rides on
            # the den psum's last column inside stage_a -- no cs matmuls.)
            for i2 in range(NCH // 2):
                tp = ph3b.tile([128, 2, HPC, 32], F32, tag="tp")
                for di in range(2):
                    i = 2 * i2 + di
                    ch = slice(T * i, T * i + T)
                    for m in range(2):
                        nc.tensor.matmul(
                            tp[:, di, 2 * m:2 * m + 2, :], kT[m][:, ch],
                            qcb_t[:, m, :], start=True, stop=True)
                nc.scalar.activation(
                    out=wT_pad[:, 2 * i2:2 * i2 + 2], in_=tp, func=EXP)

        # ---------- G = (k^T q) o U for all chunks (keeps PE dense) ----------
        gmt_all = gstore.tile([128, NCH, 4 * T], BF16, tag="gmt")
        with tc.tile_pool(name="pg", bufs=4, space="PSUM") as pg:
            for i in range(NCH):
                ch = slice(T * i, T * i + T)
                gt = pg.tile([128, 512], F32, tag="gt")
                for m in range(2):
                    nc.tensor.matmul(
                        gt[:, 256 * m:256 * m + 256],
                        kT[m][:, ch], qblk[m][:, i, :], start=True, stop=True)
                nc.vector.tensor_mul(gmt_all[:, i, :], gt, u4_t)

        with tc.tile_pool(name="ph3c", bufs=4, space="PSUM") as ph3c:
            # SK/SV chunk deltas + exclusive prefixes (slot i = state
            # before chunk i; slot 0 unused). Deltas stay per-head: the
            # prefix chain then needs only TWO wide DVE adds per boundary
            # (DVE is the scarce engine here -- only it can read PSUM for
            # tensor+tensor).
            for i in range(NCH - 1):
                skd = ph3c.tile([64, HPC, 32], F32, tag="skd")
                svd = ph3c.tile([C, HPC, DH], F32, tag="svd")
                for h in range(HPC):
                    nc.tensor.matmul(
                        skd[:, h, :],
                        knat[h // 2][:, i, 64 * (h % 2):64 * (h % 2) + 64],
                        wT_pad[:, i, h, :], start=True, stop=True)
                    nc.tensor.matmul(
                        svd[:, h, :], wT_pad[:, i, h, 0:C],
                        vnat[:, i, 64 * h:64 * h + 64], start=True, stop=True)
                # prefix adds stay WIDE and packed on DVE (2 ops, short
                # serial chain); the idle Pool fans sk out to the
                # block-diagonal layout OFF the chain.
                if i == 0:
                    nc.scalar.copy(out=sk_all[:, 1], in_=skd)
                    nc.scalar.copy(out=sv_all[:, 1], in_=svd)
                else:
                    nc.vector.tensor_add(sk_all[:, i + 1], skd, sk_all[:, i])
                    nc.vector.tensor_add(sv_all[:, i + 1], svd, sv_all[:, i])
                # fan-out on the idle Pool engine, off the DVE prefix
                # chain. Slots 10+ are deferred into the main loop: the
                # loop's psum-pool-open markers ride the Pool queue, and a
                # long copy tail here would stall the loop start ~5us.
                if i <= 8:
                    skbd_fanout(i + 1)

        # ---------- main chunk loop, software-pipelined ----------
        # Per-engine sequencers run IN ORDER, so a stage whose inputs are
        # still in flight head-of-line-blocks everything behind it. Stage
        # the emission: C(i-2) out-proj, B(i-1) PW/mix, A(i) den/att --
        # each stage's inputs were produced >= 1 iteration earlier.
        o_nm = onum.tile([128, 2, S], BF16, tag="o_nm")
        rb_all = onum.tile([128, 2, S], BF16, tag="rb_all")

        pda = ep(tc.tile_pool(name="pda", bufs=3, space="PSUM"))
        pbig = ep(tc.tile_pool(name="pbig", bufs=3, space="PSUM"))  # pw+mr
        pop = ep(tc.tile_pool(name="pop", bufs=2, space="PSUM"))

        ctiles = {}

        def stage_a(i):
            ch = slice(T * i, T * i + T)
            # den (packed 32h+c) in ONE matmul; att numerator per head into
            # the same packed layout via tile_position column offsets. For
            # i>0 the SK_i^T q_i state term accumulates into the same psum
            # region (sk_all is precomputed, so no chunk->chunk dependency).
            da = pda.tile([128, 2, T], F32, tag="da")
            den_b = da[:, 0, :]
            an_b = da[:, 1, :]
            nc.tensor.matmul(den_b, wT_pad[:, i], u128, start=True, stop=True)
            for h in range(HPC):
                nc.tensor.matmul(
                    an_b[32 * h:32 * h + 32, :], wT_pad[:, i, h, :],
                    gmt_all[:, i, 128 * h:128 * h + 128],
                    start=True, stop=(i == 0), tile_position=(0, 32 * h))
            if i > 0:
                # block-diag SK state: one matmul per head PAIR (qT[m] is
                # the stacked moving operand), 64 packed rows at a time.
                for m in range(2):
                    nc.tensor.matmul(
                        an_b[64 * m:64 * m + 64, :], sk_bd[m][:, i, :],
                        qT[m][:, ch], start=False, stop=True,
                        tile_position=(0, 64 * m), skip_group_check=True)

            # den carry chain rides on the den psum's last column (the
            # intra-chunk cumsum's final entry IS the chunk total).
            if i < NCH - 1:
                if i == 0:
                    nc.scalar.copy(out=carry_cols[:, 1:2],
                                   in_=den_b[:, T - 1:T])
                else:
                    nc.vector.tensor_add(carry_cols[:, i + 1:i + 2],
                                         den_b[:, T - 1:T],
                                         carry_cols[:, i:i + 1])

            # softmax pieces, packed layout: 1/x as exp(-ln x), den carry
            # folded into the ln bias (per-partition column).
            lden = sb2.tile([128, T], F32, tag="lden")
            if i == 0:
                nc.scalar.activation(out=lden, in_=den_b, func=LN)
            else:
                nc.scalar.activation(out=lden, in_=den_b, func=LN,
                                     bias=carry_cols[:, i:i + 1])
            rden = sb2.tile([128, T], F32, tag="rden")
            nc.scalar.activation(out=rden, in_=lden, func=EXP, scale=-1.0)
            att = sb2.tile([128, T], F32, tag="att")
            nc.vector.tensor_mul(att, an_b, rden)
            e_b = sb2.tile([128, T], BF16, tag="e")
            nc.scalar.activation(out=e_b, in_=att, func=EXP)
            # pdd directly in the (c, h, t) matmul layout: 4 small Pool
            # muls on the real c-rows -- no packed intermediate and no DMA
            # remaps (the SP sequencer's 565ns/issue was loop-critical).
            pdd_u = sb2.tile([C, HPC, T], BF16, tag="pddu")
            for h in range(HPC):
                nc.gpsimd.tensor_mul(pdd_u[:, h, :],
                                     e_b[32 * h:32 * h + C, :],
                                     rden[32 * h:32 * h + C, :])
            ctiles[i] = (e_b, pdd_u)

        obi = [0]

        def oproj_block(c0, nch=4):
            """Normalize + out-project + store chunks [c0, c0+nch). Emitted
            >= 3 chunks behind the producers, so this is a dense matmul
            burst -- long-ready inputs, no head-of-line blocking."""
            bs = slice(T * c0, T * c0 + nch * T)
            oT_b = otp.tile([128, 2, 4 * T], BF16, tag=f"oTb{obi[0] % 2}")
            obi[0] += 1
            oT_b = oT_b[:, :, 0:nch * T]
            # all-SBUF bf16 multiply -> GpSimd (otherwise idle; DVE is loaded)
            nc.gpsimd.tensor_mul(oT_b, o_nm[:, :, bs], rb_all[:, :, bs])
            for sc in range(nch):
                ob = outp.tile([128, D], BF16, tag="ob")
                for n2 in range(2):
                    ps = pop.tile([128, 512], F32, tag="ops")
                    for kt in range(2):
                        nc.tensor.matmul(
                            ps, oT_b[:, kt, T * sc:T * sc + T],
                            woT_t[:, kt, 512 * n2:512 * n2 + 512],
                            start=(kt == 0), stop=(kt == 1))
                    if n2 == 0:
                        nc.vector.tensor_copy(
                            out=ob[:, 512 * n2:512 * n2 + 512], in_=ps)
                    else:
                        nc.scalar.copy(
                            out=ob[:, 512 * n2:512 * n2 + 512], in_=ps)
                nc.sync.dma_start(
                    out=d["out_d"].ap()[T * (c0 + sc):T * (c0 + sc) + T, :],
                    in_=ob)

        def stage_b(i):
            ch = slice(T * i, T * i + T)
            e_b, pdd_u = ctiles[i]
            # PW (t, s) + mask
            pw = pbig.tile([128, 512], F32, tag="big")
            for h in range(HPC):
                nc.tensor.matmul(
                    pw[:, 128 * h:128 * h + 128], w16[:, h, ch],
                    pdd_u[:, h, :], start=True, stop=True)
            pwm = sb2.tile([128, 512], BF16, tag="pwm")
            nc.vector.tensor_mul(pwm, pw, u4_t)

            # o numerator (cols 0-255) + sums broadcast (cols 256-511).
            # rb and sv matmuls first: their inputs (e_b, pdd_u) landed an
            # iteration ago, so they bridge the ~790ns the PE would
            # otherwise idle waiting for the pwm mask-mul on DVE.
            mr = pbig.tile([128, 512], F32, tag="big")
            for pair in range(2):
                nc.tensor.matmul(
                    mr[:, 256 + 128 * pair:384 + 128 * pair],
                    sel2_t[:, pair, :], e_b, start=True, stop=True)
            for h in (0, 2, 1, 3):          # bp-stable order
                bp = 64 * (h % 2)
                cb = 128 * (h // 2)
                nc.tensor.matmul(
                    mr[bp:bp + 64, cb:cb + 128],
                    vnat[:, i, 64 * h:64 * h + 64],
                    pwm[:, 128 * h:128 * h + 128],
                    start=True, stop=(i == 0), tile_position=(0, bp))
                if i > 0:
                    nc.tensor.matmul(
                        mr[bp:bp + 64, cb:cb + 128],
                        sv_all[:, i, h, :], pdd_u[:, h, :],
                        start=False, stop=True, tile_position=(0, bp))
            nc.vector.tensor_copy(out=o_nm[:, :, ch], in_=mr[:, 0:256])
            lsb = sb2.tile([128, 256], F32, tag="lsb")
            nc.scalar.activation(out=lsb, in_=mr[:, 256:512], func=LN)
            nc.scalar.activation(out=rb_all[:, :, ch], in_=lsb,
                                 func=EXP, scale=-1.0)

        for i in range(NCH + 2):
            if i < 6:
                skbd_fanout(i + 10)     # deferred sk_bd fan-outs (slots
            if 2 <= i:                  # 10..15, consumed at iters 10..15)
                stage_b(i - 2)          # lag 2: A(i)'s chain has ~2
            if i < NCH:                 # iterations to land before B reads it
                stage_a(i)
            if i == 10:
                oproj_block(0)          # 3+ chunks behind the producers
            elif i == 13:
                oproj_block(4)
            elif i == 16:
                oproj_block(8)
            elif i == 17:
                oproj_block(12, 2)      # small tail blocks: the final
        oproj_block(14, 2)              # post-loop burst is ~half as long


def kernel(**inputs):
    global LAST_EXEC_NS
    x = np.ascontiguousarray(inputs["x"], np.float32)
    q_c, beta = np.asarray(inputs["q_c"]), np.asarray(inputs["beta"])
    Wq, bq = np.asarray(inputs["Wq"]), np.asarray(inputs["bq"])
    Wk, bk = np.asarray(inputs["Wk"]), np.asarray(inputs["bk"])
    Wv, bv = np.asarray(inputs["Wv"]), np.asarray(inputs["bv"])
    Wo, bo = np.asarray(inputs["Wo"]), np.asarray(inputs["bo"])

    if "nc" not in _CACHE:
        _CACHE["nc"] = _build()
    nc = _CACHE["nc"]

    BF = ml_dtypes.bfloat16
    # per-head query temperature 0.125*exp(-beta) folded into Wq/bq
    qscale = (0.125 * np.exp(-beta.astype(np.float64))).astype(np.float32)
    qs_hd = np.repeat(qscale, DH)                      # (D,) per out-dim
    Wq_s = Wq * qs_hd[:, None]
    bq_s = bq * qs_hd

    in_maps = []
    for core in range(8):
        b, hh = core // 4, core % 4
        hd = slice(hh * HD, hh * HD + HD)
        # qc block-diagonal per head pair, matching kT[m]'s [h_2m; h_2m+1]
        # partition stacking; zero c-pad 16->32.
        qcb = np.zeros((2, 128, 64), BF)
        qc_r = q_c[:, hd].reshape(C, HPC, DH)          # (c, h, d)
        for m in range(2):
            qcb[m, 0:64, 0:C] = qc_r[:, 2 * m, :].T.astype(BF)
            qcb[m, 64:128, 32:32 + C] = qc_r[:, 2 * m + 1, :].T.astype(BF)
        # pre-shuffles: one contiguous DRAM run per SBUF partition line.
        # xT (D, S) -> (p, nblk, kt, s'); w*T (D, HD) -> (p, kt, j);
        # woT (HD, D) -> (p, kt2, j).
        def wshuf(w):          # (D, HD) -> (128, KT, HD)
            return np.ascontiguousarray(
                w.reshape(KT, 128, HD).transpose(1, 0, 2).astype(BF))
        xT = x[:, b, :].T.astype(BF)                    # (D, S)
        xsh = np.ascontiguousarray(
            xT.reshape(KT, 128, 4, 512).transpose(1, 2, 0, 3))
        in_maps.append({
            "xT": xsh,
            "wqT": wshuf(Wq_s[hd, :].T),
            "wkT": wshuf(Wk[hd, :].T),
            "wvT": wshuf(Wv[hd, :].T),
            "woT": np.ascontiguousarray(
                Wo[:, hd].T.reshape(2, 128, D).transpose(1, 0, 2).astype(BF)),
            "qcb": qcb,
            "bq": np.ascontiguousarray(bq_s[hd].reshape(2, 128).T),
            "bk": np.ascontiguousarray(bk[hd].reshape(2, 128).T),
        })

    trace = os.environ.get("TRN_PROFILE", "0") == "1"
    res = run_bass_kernel_spmd(nc, in_maps, list(range(8)), trace=trace)
    LAST_EXEC_NS = res.exec_time_ns

    out = np.zeros((S, B, D), np.float32)
    for core in range(8):
        out[:, core // 4, :] += res.results[core]["out_p"].astype(np.float32)
    # v-bias folded out of the kernel: softmax weights sum to 1, so +bv
    # passes through both attention levels and lands as a constant Wo@bv.
    out += (bo + Wo @ bv.astype(np.float32))[None, None, :]
    return out



# revision 63
# speedup vs baseline: 1.0445x; 1.0382x over previous
"""CompressionAttention Trainium2 kernel (8 NeuronCores, SPMD).

Sharding: core i handles batch b=i//4 and 4 heads hh=i%4 (model-dim slice
hh*256:(hh+1)*256). Heads never interact before out_proj, so each core
computes a partial out-projection for its batch; the host gather sums the
4 partials per batch and adds bo once.

Algorithm per core (chunked linear attention, chunk T=128):
  w[c,t] = exp(qc_c . k_t)            (max-subtraction dropped: att is
                                       invariant to per-c scaling of w)
  den[c,s]   = cumsum_t<=s w[c,t]
  att[c,s]   = (w_chunk^T (U o G) + SK^T qs) / den   per chunk + state
  p = softmax_c att ; o_s = sum_c p * (sum_t<=s w v)/den

Schedule: all cross-chunk recurrences (den carry, SK/SV prefixes) and all
G = k^T q blocks are precomputed in side phases, so the main chunk loop has
no chunk->chunk dependency and the Tensor engine never idles long enough to
HAM-throttle. Softmax elementwise ops run in a partition-packed (32h+c, t)
layout (engine time ~ free-size, so 4x cheaper than (c, 4h*t)); the packed
e/pdd are re-laid-out for their matmul consumers with tiny SBUF-to-SBUF
DMAs. 1/x runs as exp(-ln(x)) on Act (single act table, den carry folded
into the ln bias); 1/sums o-normalization is deferred out of the loop.
"""
import os
import numpy as np
import ml_dtypes

import concourse.bacc as bacc
import concourse.tile as tile
from concourse import mybir
from concourse.bass_utils import run_bass_kernel_spmd

F32 = mybir.dt.float32
BF16 = mybir.dt.bfloat16
EXP = mybir.ActivationFunctionType.Exp
LN = mybir.ActivationFunctionType.Ln
IDENT = mybir.ActivationFunctionType.Identity

S, B, D = 2048, 2, 1024
H, DH, C = 16, 64, 16
HPC = 4            # heads per core
HD = HPC * DH      # 256 model dims per core
T = 128            # chunk
NCH = S // T       # 16 chunks
KT = D // 128      # 8 k-tiles of the model dim

LAST_EXEC_NS = None
_CACHE = {}


def _patched_insert_act_table_loads(self):
    """Force every activation (Exp/Ln/Copy/Identity) onto the single
    act-func table that contains them all. The default greedy pass picks
    exp_and_others for Exp and natural_log_exp_and_others for Ln, inserting
    an ACT_TABLE_LOAD (1.28us!) on every switch."""
    import bass_rust as _bass_rust
    from concourse.hw_specs import get_activation_tables
    has_activation = any(
        isinstance(i, mybir.InstActivation)
        for b in self.main_func.blocks
        for i in b.instructions)
    if not has_activation:
        return
    keep = {mybir.ActivationFunctionType.Exp, mybir.ActivationFunctionType.Ln,
            mybir.ActivationFunctionType.Copy,
            mybir.ActivationFunctionType.Identity}
    tables = []
    for name, funcs in get_activation_tables(self.m.arch).items():
        if name != "natural_log_exp_and_others":
            funcs = funcs - keep
        tables.append((name, funcs))
    _bass_rust.insert_act_table_loads(self, tables)


def _build():
    nc = bacc.Bacc("TRN2", target_bir_lowering=False, debug=False, num_devices=8)
    import types as _types
    nc.insert_act_table_loads = _types.MethodType(
        _patched_insert_act_table_loads, nc)

    # ---- DRAM I/O (per-core, host-prepped layouts; bf16 where possible).
    # x and weights are pre-shuffled on host so each SBUF partition line is
    # ONE contiguous DRAM run (8KB / 4KB descriptors -> near-peak DMA bw;
    # the natural (kt p) layouts gave 512B-1KB descriptors, ~3x slower). ----
    xT_d = nc.dram_tensor("xT", [128, 4, KT, 512], BF16, kind="ExternalInput")
    wqT_d = nc.dram_tensor("wqT", [128, KT, HD], BF16, kind="ExternalInput")
    wkT_d = nc.dram_tensor("wkT", [128, KT, HD], BF16, kind="ExternalInput")
    wvT_d = nc.dram_tensor("wvT", [128, KT, HD], BF16, kind="ExternalInput")
    woT_d = nc.dram_tensor("woT", [128, 2, D], BF16, kind="ExternalInput")
    # qc block-diagonal per head PAIR: rows 0-63 = head 2m's d, rows 64-127 =
    # head 2m+1's d (matching the [k_2m; k_2m+1] stacking of kT[m]), cols =
    # (j, c) with zero off-blocks and zero c-pad 16->32 (pad logits are 0 ->
    # w=1, never read). One matmul then emits BOTH heads' wT/w16 block.
    qcb_d = nc.dram_tensor("qcb", [2, 128, 64], BF16, kind="ExternalInput")
    # biases host-prepped as (p, m) so each partition line is one contiguous
    # 8B descriptor (the (m p)->p m rearrange of a flat [256] makes 256
    # 4-byte descriptors).
    bq_d = nc.dram_tensor("bq", [128, 2], F32, kind="ExternalInput")
    bk_d = nc.dram_tensor("bk", [128, 2], F32, kind="ExternalInput")
    out_d = nc.dram_tensor("out_p", [S, D], BF16, kind="ExternalOutput")

    # ---- consts baked into the NEFF ----
    u = np.triu(np.ones((T, T), np.float32))
    u4 = np.tile(u, (1, 4)).astype(ml_dtypes.bfloat16)
    u4_d = nc.inline_tensor(u4, "u4c")
    # sel2[pair][p, j] = 1 iff p selects head 2*pair (j<64) / 2*pair+1
    # (j>=64): the sums-broadcast matmul consumes packed (32h+c) e directly
    # and emits both heads of a column-block in one matmul.
    sel2 = np.zeros((2, 128, 128), np.float32)
    for pair in range(2):
        sel2[pair, 64 * pair:64 * pair + C, 0:64] = 1.0
        sel2[pair, 64 * pair + 32:64 * pair + 32 + C, 64:128] = 1.0
    sel2_d = nc.inline_tensor(
        np.ascontiguousarray(sel2.transpose(1, 0, 2)).astype(ml_dtypes.bfloat16),
        "sel2")
    kv_scratch = [nc.dram_tensor(f"kvs{j}", [128, 2048], BF16) for j in range(2)]

    with tile.TileContext(nc) as tc:
        _emit(nc, tc, locals())
    nc.compile()
    return nc


def _emit(nc, tc, d):
    from contextlib import ExitStack

    with ExitStack() as ctx:
        ep = ctx.enter_context

        # ---------- persistent pools ----------
        consts = ep(tc.tile_pool(name="consts", bufs=1))
        wpool = ep(tc.tile_pool(name="wpool", bufs=1))      # WoT
        qkv = ep(tc.tile_pool(name="qkv", bufs=1))          # qT/kT/vnat
        qkv2 = ep(tc.tile_pool(name="qkv2", bufs=1))        # knat, row-0 copies
        wstore = ep(tc.tile_pool(name="wstore", bufs=1))    # w16/wT/carry/sk/sv
        gstore = ep(tc.tile_pool(name="gstore", bufs=1))    # masked G, all chunks
        onum = ep(tc.tile_pool(name="onum", bufs=1))        # o_num, rb_all
        sb2 = ep(tc.tile_pool(name="sb2", bufs=3))          # per-chunk sbuf
        otp = ep(tc.tile_pool(name="otp", bufs=1))          # oT final
        outp = ep(tc.tile_pool(name="outp", bufs=3))        # out staging

        # ---------- consts (issued on the Act HWDGE sequencer so the SP
        # sequencer's 565ns/issue budget is spent only on the big x/weight
        # loads; Act is idle during the load phase anyway) ----------
        def bias_tile(name):
            t = consts.tile([128, 2], F32, tag=name)
            nc.scalar.dma_start(out=t, in_=d[name + "_d"].ap())
            return t
        bk_t, bq_t = bias_tile("bk"), bias_tile("bq")
        u4_t = consts.tile([128, 4 * T], BF16, tag="u4")
        nc.scalar.dma_start(out=u4_t, in_=d["u4_d"].ap())
        u128 = u4_t[:, 0:T]
        sel2_t = consts.tile([128, 2, 128], BF16, tag="sel2")
        nc.scalar.dma_start(out=sel2_t, in_=d["sel2_d"].ap())
        qcb_t = consts.tile([128, 2, 64], BF16, tag="qcb")
        nc.scalar.dma_start(
            out=qcb_t, in_=d["qcb_d"].ap().rearrange("m p j -> p m j"))

        # WoT resident (bf16): (128, 2, D); only needed at the first oproj
        # block mid-loop, so it loads on Act after the consts.
        woT_t = wpool.tile([128, 2, D], BF16, tag="woT")
        nc.scalar.dma_start(out=woT_t, in_=d["woT_d"].ap())

        # ---------- projections (all bf16) ----------
        qT = [qkv.tile([128, S], BF16, tag=f"qT{m}", name=f"qT{m}") for m in range(2)]
        kT = [qkv.tile([128, S], BF16, tag=f"kT{m}", name=f"kT{m}") for m in range(2)]
        vnat = qkv.tile([128, NCH, HD], BF16, tag="vnat", name="vnat")

        with tc.tile_pool(name="xw", bufs=1) as xw, \
                tc.tile_pool(name="ppj", bufs=4, space="PSUM") as ppj:
            # DMA order = consumption order, ONE dma_start per logical block
            # (descriptors spray across all 16 DMA engines, so a block moves
            # at aggregate bandwidth; splitting it only costs SP issue slots
            # at 565ns each). x goes s-block-major: the n-outer wave below
            # consumes 512-seq-col blocks, so the PE starts after wk + the
            # FIRST 1MB of x (~5us) instead of the whole 4MB (~24us).
            xT_t = xw.tile([128, 4, KT, 512], BF16, tag="xT")
            wts = {}
            for nm in ("wk", "wq", "wv"):
                wts[nm] = xw.tile([128, KT, HD], BF16, tag=nm, name=nm)
            xr = d["xT_d"].ap()
            nc.sync.dma_start(out=wts["wk"], in_=d["wkT_d"].ap())
            nc.sync.dma_start(out=xT_t[:, 0], in_=xr[:, 0])
            nc.sync.dma_start(out=xT_t[:, 1], in_=xr[:, 1])
            nc.sync.dma_start(out=wts["wq"], in_=d["wqT_d"].ap())
            nc.sync.dma_start(out=xT_t[:, 2], in_=xr[:, 2])
            nc.sync.dma_start(out=xT_t[:, 3], in_=xr[:, 3])
            nc.sync.dma_start(out=wts["wv"], in_=d["wvT_d"].ap())

            knat = [qkv2.tile([128, NCH, 128], BF16, tag=f"knat{m}",
                              name=f"knat{m}") for m in range(2)]
            kvs = d["kv_scratch"]
            # block-diagonal q for the pair-merged G matmuls: built by the
            # otherwise-idle Pool engine from the q epilogues (off-blocks
            # zeroed up front, also on Pool).
            qblk = [qkv2.tile([128, NCH, 256], BF16, tag=f"qblk{m}",
                              name=f"qblk{m}") for m in range(2)]
            for m in range(2):
                nc.gpsimd.memset(qblk[m][:, :, :], 0.0)

            # n-OUTER waves: psum (n, m) accumulates its 8 kt matmuls
            # back-to-back and needs only x s-block n, so wave 0 runs as
            # soon as the first 1MB of x lands and later waves chase the
            # DMA stream (1MB every ~2.8us vs 3.4us of PE work per block).
            epi = 0

            def qk_waves(nm, dst, bias, blkdst=None):
                nonlocal epi
                for n in range(4):
                    for m in range(2):
                        ps = ppj.tile([128, 512], F32, tag="pj",
                                      name=f"pj{n}{m}")
                        for kt in range(KT):
                            nc.tensor.matmul(
                                ps, wts[nm][:, kt, 128 * m:128 * m + 128],
                                xT_t[:, n, kt, :],
                                start=(kt == 0), stop=(kt == KT - 1))
                        sl = slice(512 * n, 512 * n + 512)
                        if epi % 2 == 0:
                            nc.vector.tensor_scalar(
                                out=dst[m][:, sl], in0=ps,
                                scalar1=bias[:, m:m + 1], scalar2=None,
                                op0=mybir.AluOpType.add)
                        else:
                            nc.scalar.activation(
                                out=dst[m][:, sl], in_=ps,
                                func=IDENT, bias=bias[:, m:m + 1])
                        if blkdst is not None:
                            src = dst[m][:, sl].rearrange(
                                "p (i t) -> p i t", i=4)
                            nc.gpsimd.tensor_copy(
                                out=blkdst[m][0:64, 4 * n:4 * n + 4, 0:128],
                                in_=src[0:64])
                            nc.gpsimd.tensor_copy(
                                out=blkdst[m][64:128, 4 * n:4 * n + 4,
                                              128:256],
                                in_=src[64:128])
                        epi += 1

            # k first, then q, then v
            qk_waves("wk", kT, bk_t)
            # kick the k natural-layout DRAM round-trip early so the
            # xbar transpose lands while v projects
            for m in range(2):
                nc.sync.dma_start(out=kvs[m].ap(), in_=kT[m][:, :])
                nc.sync.dma_start_transpose(out=knat[m], in_=kvs[m].ap())
            qk_waves("wq", qT, bq_t, blkdst=qblk)

            # v directly in natural (s, hd) layout: out = xT.T @ wvT. No
            # bias matmul: softmax weights sum to 1, so +bv passes straight
            # through both attention levels; the host folds Wo@bv into bo.
            for wv in range(4):
                vps = [ppj.tile([128, HD], F32, tag="vp", name=f"vp{j}")
                       for j in range(4)]
                for kt in range(KT):
                    for j in range(4):
                        sc = 4 * wv + j
                        nc.tensor.matmul(
                            vps[j],
                            xT_t[:, sc // 4, kt,
                                 T * (sc % 4):T * (sc % 4) + T],
                            wts["wv"][:, kt, :],
                            start=(kt == 0), stop=(kt == KT - 1))
                for j in range(4):
                    sc = 4 * wv + j
                    if sc % 2 == 0:
                        nc.vector.tensor_copy(out=vnat[:, sc, :], in_=vps[j])
                    else:
                        nc.scalar.copy(out=vnat[:, sc, :], in_=vps[j])

        # ---------- w16, packed wT, den carries, SK/SV prefixes ----------
        w16 = wstore.tile([C, HPC, S], BF16, tag="w16")
        # wT packed (t, 32h+c): cols 16-31 of each 32-group hold exp(0)=1
        # from the zero-padded qc -- written, never read back.
        wT_pad = wstore.tile([128, NCH, HPC, 32], BF16, tag="wTp")
        carry_cols = wstore.tile([128, NCH], F32, tag="carryc")
        # SK prefix state, BLOCK-DIAGONAL per head pair: partitions (j, d)
        # stack heads 2m/2m+1 exactly like kT[m]/qT[m], cols (j', c) hold
        # head 2m+j' only on the diagonal j'==j (off-blocks zeroed once by
        # the idle Pool engine). stage_a then needs only TWO SK^T q matmuls
        # per chunk -- qT[m] is already the pair-stacked moving operand.
        sk_bd = [wstore.tile([128, NCH, 64], BF16, tag=f"skbd{m}",
                             name=f"skbd{m}")
                 for m in range(2)]
        for m in range(2):
            nc.gpsimd.memset(sk_bd[m][:, :, :], 0.0)
        sk_all = wstore.tile([64, NCH, HPC, 32], BF16, tag="skal")
        sv_all = wstore.tile([C, NCH, HPC, DH], BF16, tag="sv")

        def skbd_fanout(s):
            for h in range(HPC):
                m, j = h // 2, h % 2
                nc.gpsimd.tensor_copy(
                    out=sk_bd[m][64 * j:64 * j + 64, s, 32 * j:32 * j + 32],
                    in_=sk_all[:, s, h, :])

        w16p = gstore.tile([128, S], BF16, tag="w16p")
        with tc.tile_pool(name="ph3a", bufs=3, space="PSUM") as ph3a:
            # w16 = exp(qc . k): the block-diag qcb emits BOTH heads of a
            # pair per matmul (kT[m] stacks them on 128 partitions), ONE exp
            # per 512-slice, then 4 sbuf-to-sbuf DMAs remap rows to the
            # (c, h, s) layout.
            for n in range(4):
                ps = ph3a.tile([128, 512], F32, tag="w16ps")
                for m in range(2):
                    nc.tensor.matmul(
                        ps[64 * m:64 * m + 64, :], qcb_t[:, m, :],
                        kT[m][:, 512 * n:512 * n + 512],
                        start=True, stop=True, tile_position=(0, 64 * m))
                nc.scalar.activation(
                    out=w16p[:, 512 * n:512 * n + 512], in_=ps, func=EXP)
            for h in range(HPC):
                nc.sync.dma_start(out=w16[:, h, :],
                                  in_=w16p[32 * h:32 * h + C, :])

        with tc.tile_pool(name="ph3b", bufs=4, space="PSUM") as ph3b:
            # wT (t, 32h+c) = exp(k . qc_pad): 2 matmuls/chunk via the
            # stacked-pair trick, exp batched over chunk PAIRS so the Act
            # queue doesn't trail the PE. (The den carry chain now rides on
            # the den psum's last column inside stage_a -- no cs matmuls.)
            for i2 in range(NCH // 2):
                tp = ph3b.tile([128, 2, HPC, 32], F32, tag="tp")
                for di in range(2):
                    i = 2 * i2 + di
                    ch = slice(T * i, T * i + T)
                    for m in range(2):
                        nc.tensor.matmul(
                            tp[:, di, 2 * m:2 * m + 2, :], kT[m][:, ch],
                            qcb_t[:, m, :], start=True, stop=True)
                nc.scalar.activation(
                    out=wT_pad[:, 2 * i2:2 * i2 + 2], in_=tp, func=EXP)

        # ---------- G = (k^T q) o U for all chunks (keeps PE dense) ----------
        gmt_all = gstore.tile([128, NCH, 4 * T], BF16, tag="gmt")
        with tc.tile_pool(name="pg", bufs=4, space="PSUM") as pg:
            for i in range(NCH):
                ch = slice(T * i, T * i + T)
                gt = pg.tile([128, 512], F32, tag="gt")
                for m in range(2):
                    nc.tensor.matmul(
                        gt[:, 256 * m:256 * m + 256],
                        kT[m][:, ch], qblk[m][:, i, :], start=True, stop=True)
                nc.vector.tensor_mul(gmt_all[:, i, :], gt, u4_t)

        with tc.tile_pool(name="ph3c", bufs=4, space="PSUM") as ph3c:
            # SK/SV chunk deltas + exclusive prefixes (slot i = state
            # before chunk i; slot 0 unused). Deltas stay per-head: the
            # prefix chain then needs only TWO wide DVE adds per boundary
            # (DVE is the scarce engine here -- only it can read PSUM for
            # tensor+tensor).
            for i in range(NCH - 1):
                skd = ph3c.tile([64, HPC, 32], F32, tag="skd")
                svd = ph3c.tile([C, HPC, DH], F32, tag="svd")
                for h in range(HPC):
                    nc.tensor.matmul(
                        skd[:, h, :],
                        knat[h // 2][:, i, 64 * (h % 2):64 * (h % 2) + 64],
                        wT_pad[:, i, h, :], start=True, stop=True)
                    nc.tensor.matmul(
                        svd[:, h, :], wT_pad[:, i, h, 0:C],
                        vnat[:, i, 64 * h:64 * h + 64], start=True, stop=True)
                # prefix adds stay WIDE and packed on DVE (2 ops, short
                # serial chain); the idle Pool fans sk out to the
                # block-diagonal layout OFF the chain.
                if i == 0:
                    nc.scalar.copy(out=sk_all[:, 1], in_=skd)
                    nc.scalar.copy(out=sv_all[:, 1], in_=svd)
                else:
                    nc.vector.tensor_add(sk_all[:, i + 1], skd, sk_all[:, i])
                    nc.vector.tensor_add(sv_all[:, i + 1], svd, sv_all[:, i])
                # fan-out on the idle Pool engine, off the DVE prefix
                # chain. Slots 10+ are deferred into the main loop: the
                # loop's psum-pool-open markers ride the Pool queue, and a
                # long copy tail here would stall the loop start ~5us.
                if i <= 8:
                    skbd_fanout(i + 1)

        # ---------- main chunk loop, software-pipelined ----------
        # Per-engine sequencers run IN ORDER, so a stage whose inputs are
        # still in flight head-of-line-blocks everything behind it. Stage
        # the emission: C(i-2) out-proj, B(i-1) PW/mix, A(i) den/att --
        # each stage's inputs were produced >= 1 iteration earlier.
        o_nm = onum.tile([128, 2, S], BF16, tag="o_nm")
        rb_all = onum.tile([128, 2, S], BF16, tag="rb_all")

        pda = ep(tc.tile_pool(name="pda", bufs=3, space="PSUM"))
        pbig = ep(tc.tile_pool(name="pbig", bufs=3, space="PSUM"))  # pw+mr
        pop = ep(tc.tile_pool(name="pop", bufs=2, space="PSUM"))

        ctiles = {}

        def stage_a(i):
            ch = slice(T * i, T * i + T)
            # den (packed 32h+c) in ONE matmul; att numerator per head into
            # the same packed layout via tile_position column offsets. For
            # i>0 the SK_i^T q_i state term accumulates into the same psum
            # region (sk_all is precomputed, so no chunk->chunk dependency).
            da = pda.tile([128, 2, T], F32, tag="da")
            den_b = da[:, 0, :]
            an_b = da[:, 1, :]
            nc.tensor.matmul(den_b, wT_pad[:, i], u128, start=True, stop=True)
            for h in range(HPC):
                nc.tensor.matmul(
                    an_b[32 * h:32 * h + 32, :], wT_pad[:, i, h, :],
                    gmt_all[:, i, 128 * h:128 * h + 128],
                    start=True, stop=(i == 0), tile_position=(0, 32 * h))
            if i > 0:
                # block-diag SK state: one matmul per head PAIR (qT[m] is
                # the stacked moving operand), 64 packed rows at a time.
                for m in range(2):
                    nc.tensor.matmul(
                        an_b[64 * m:64 * m + 64, :], sk_bd[m][:, i, :],
                        qT[m][:, ch], start=False, stop=True,
                        tile_position=(0, 64 * m), skip_group_check=True)

            # den carry chain rides on the den psum's last column (the
            # intra-chunk cumsum's final entry IS the chunk total).
            if i < NCH - 1:
                if i == 0:
                    nc.scalar.copy(out=carry_cols[:, 1:2],
                                   in_=den_b[:, T - 1:T])
                else:
                    nc.vector.tensor_add(carry_cols[:, i + 1:i + 2],
                                         den_b[:, T - 1:T],
                                         carry_cols[:, i:i + 1])

            # softmax pieces, packed layout: 1/x as exp(-ln x), den carry
            # folded into the ln bias (per-partition column).
            lden = sb2.tile([128, T], F32, tag="lden")
            if i == 0:
                nc.scalar.activation(out=lden, in_=den_b, func=LN)
            else:
                nc.scalar.activation(out=lden, in_=den_b, func=LN,
                                     bias=carry_cols[:, i:i + 1])
            rden = sb2.tile([128, T], F32, tag="rden")
            nc.scalar.activation(out=rden, in_=lden, func=EXP, scale=-1.0)
            att = sb2.tile([128, T], F32, tag="att")
            nc.vector.tensor_mul(att, an_b, rden)
            e_b = sb2.tile([128, T], BF16, tag="e")
            nc.scalar.activation(out=e_b, in_=att, func=EXP)
            # pdd directly in the (c, h, t) matmul layout: 4 small Pool
            # muls on the real c-rows -- no packed intermediate and no DMA
            # remaps (the SP sequencer's 565ns/issue was loop-critical).
            pdd_u = sb2.tile([C, HPC, T], BF16, tag="pddu")
            for h in range(HPC):
                nc.gpsimd.tensor_mul(pdd_u[:, h, :],
                                     e_b[32 * h:32 * h + C, :],
                                     rden[32 * h:32 * h + C, :])
            ctiles[i] = (e_b, pdd_u)

        obi = [0]

        def oproj_block(c0, nch=4):
            """Normalize + out-project + store chunks [c0, c0+nch). Emitted
            >= 3 chunks behind the producers, so this is a dense matmul
            burst -- long-ready inputs, no head-of-line blocking."""
            bs = slice(T * c0, T * c0 + nch * T)
            oT_b = otp.tile([128, 2, 4 * T], BF16, tag=f"oTb{obi[0] % 2}")
            obi[0] += 1
            oT_b = oT_b[:, :, 0:nch * T]
            # all-SBUF bf16 multiply -> GpSimd (otherwise idle; DVE is loaded)
            nc.gpsimd.tensor_mul(oT_b, o_nm[:, :, bs], rb_all[:, :, bs])
            for sc in range(nch):
                ob = outp.tile([128, D], BF16, tag="ob")
                for n2 in range(2):
                    ps = pop.tile([128, 512], F32, tag="ops")
                    for kt in range(2):
                        nc.tensor.matmul(
                            ps, oT_b[:, kt, T * sc:T * sc + T],
                            woT_t[:, kt, 512 * n2:512 * n2 + 512],
                            start=(kt == 0), stop=(kt == 1))
                    if n2 == 0:
                        nc.vector.tensor_copy(
                            out=ob[:, 512 * n2:512 * n2 + 512], in_=ps)
                    else:
                        nc.scalar.copy(
                            out=ob[:, 512 * n2:512 * n2 + 512], in_=ps)
                nc.sync.dma_start(
                    out=d["out_d"].ap()[T * (c0 + sc):T * (c0 + sc) + T, :],
                    in_=ob)

        def stage_b(i):
            ch = slice(T * i, T * i + T)
            e_b, pdd_u = ctiles[i]
            # PW (t, s) + mask
            pw = pbig.tile([128, 512], F32, tag="big")
            for h in range(HPC):
                nc.tensor.matmul(
                    pw[:, 128 * h:128 * h + 128], w16[:, h, ch],
                    pdd_u[:, h, :], start=True, stop=True)
            pwm = sb2.tile([128, 512], BF16, tag="pwm")
            nc.vector.tensor_mul(pwm, pw, u4_t)

            # o numerator (cols 0-255) + sums broadcast (cols 256-511).
            # rb and sv matmuls first: their inputs (e_b, pdd_u) landed an
            # iteration ago, so they bridge the ~790ns the PE would
            # otherwise idle waiting for the pwm mask-mul on DVE.
            mr = pbig.tile([128, 512], F32, tag="big")
            for pair in range(2):
                nc.tensor.matmul(
                    mr[:, 256 + 128 * pair:384 + 128 * pair],
                    sel2_t[:, pair, :], e_b, start=True, stop=True)
            for h in (0, 2, 1, 3):          # bp-stable order
                bp = 64 * (h % 2)
                cb = 128 * (h // 2)
                nc.tensor.matmul(
                    mr[bp:bp + 64, cb:cb + 128],
                    vnat[:, i, 64 * h:64 * h + 64],
                    pwm[:, 128 * h:128 * h + 128],
                    start=True, stop=(i == 0), tile_position=(0, bp))
                if i > 0:
                    nc.tensor.matmul(
                        mr[bp:bp + 64, cb:cb + 128],
                        sv_all[:, i, h, :], pdd_u[:, h, :],
                        start=False, stop=True, tile_position=(0, bp))
            nc.vector.tensor_copy(out=o_nm[:, :, ch], in_=mr[:, 0:256])
            lsb = sb2.tile([128, 256], F32, tag="lsb")
            nc.scalar.activation(out=lsb, in_=mr[:, 256:512], func=LN)
            nc.scalar.activation(out=rb_all[:, :, ch], in_=lsb,
                                 func=EXP, scale=-1.0)

        for i in range(NCH + 2):
            if 2 <= i:
                stage_b(i - 2)          # lag 2: A(i)'s chain has ~2
            if i < NCH:                 # iterations to land before B reads it
                stage_a(i)
            if i < 6:
                skbd_fanout(i + 10)     # deferred sk_bd fan-outs (slots
                                        # 10..15, consumed at iters 10..15)
            if i == 10:
                oproj_block(0)          # 3+ chunks behind the producers
            elif i == 13:
                oproj_block(4)
            elif i == 16:
                oproj_block(8)
            elif i == 17:
                oproj_block(12, 2)      # small tail blocks: the final
        oproj_block(14, 2)              # post-loop burst is ~half as long


def kernel(**inputs):
    global LAST_EXEC_NS
    x = np.ascontiguousarray(inputs["x"], np.float32)
    q_c, beta = np.asarray(inputs["q_c"]), np.asarray(inputs["beta"])
    Wq, bq = np.asarray(inputs["Wq"]), np.asarray(inputs["bq"])
    Wk, bk = np.asarray(inputs["Wk"]), np.asarray(inputs["bk"])
    Wv, bv = np.asarray(inputs["Wv"]), np.asarray(inputs["bv"])
    Wo, bo = np.asarray(inputs["Wo"]), np.asarray(inputs["bo"])

    if "nc" not in _CACHE:
        _CACHE["nc"] = _build()
    nc = _CACHE["nc"]

    BF = ml_dtypes.bfloat16
    # per-head query temperature 0.125*exp(-beta) folded into Wq/bq
    qscale = (0.125 * np.exp(-beta.astype(np.float64))).astype(np.float32)
    qs_hd = np.repeat(qscale, DH)                      # (D,) per out-dim
    Wq_s = Wq * qs_hd[:, None]
    bq_s = bq * qs_hd

    in_maps = []
    for core in range(8):
        b, hh = core // 4, core % 4
        hd = slice(hh * HD, hh * HD + HD)
        # qc block-diagonal per head pair, matching kT[m]'s [h_2m; h_2m+1]
        # partition stacking; zero c-pad 16->32.
        qcb = np.zeros((2, 128, 64), BF)
        qc_r = q_c[:, hd].reshape(C, HPC, DH)          # (c, h, d)
        for m in range(2):
            qcb[m, 0:64, 0:C] = qc_r[:, 2 * m, :].T.astype(BF)
            qcb[m, 64:128, 32:32 + C] = qc_r[:, 2 * m + 1, :].T.astype(BF)
        # pre-shuffles: one contiguous DRAM run per SBUF partition line.
        # xT (D, S) -> (p, nblk, kt, s'); w*T (D, HD) -> (p, kt, j);
        # woT (HD, D) -> (p, kt2, j).
        def wshuf(w):          # (D, HD) -> (128, KT, HD)
            return np.ascontiguousarray(
                w.reshape(KT, 128, HD).transpose(1, 0, 2).astype(BF))
        xT = x[:, b, :].T.astype(BF)                    # (D, S)
        xsh = np.ascontiguousarray(
            xT.reshape(KT, 128, 4, 512).transpose(1, 2, 0, 3))
        in_maps.append({
            "xT": xsh,
            "wqT": wshuf(Wq_s[hd, :].T),
            "wkT": wshuf(Wk[hd, :].T),
            "wvT": wshuf(Wv[hd, :].T),
            "woT": np.ascontiguousarray(
                Wo[:, hd].T.reshape(2, 128, D).transpose(1, 0, 2).astype(BF)),
            "qcb": qcb,
            "bq": np.ascontiguousarray(bq_s[hd].reshape(2, 128).T),
            "bk": np.ascontiguousarray(bk[hd].reshape(2, 128).T),
        })

    trace = os.environ.get("TRN_PROFILE", "0") == "1"
    res = run_bass_kernel_spmd(nc, in_maps, list(range(8)), trace=trace)
    LAST_EXEC_NS = res.exec_time_ns

    out = np.zeros((S, B, D), np.float32)
    for core in range(8):
        out[:, core // 4, :] += res.results[core]["out_p"].astype(np.float32)
    # v-bias folded out of the kernel: softmax weights sum to 1, so +bv
    # passes through both attention levels and lands as a constant Wo@bv.
    out += (bo + Wo @ bv.astype(np.float32))[None, None, :]
    return out

